# revision 2
# baseline (speedup 1.0000x reference)
"""2-hop GCN on 8 trn2 cores — GPSIMD ap_gather architecture.

out = log_softmax((D^-.5 (A+I) D^-.5)^2 x W + b)

Device layout (feature slot f = ch + 16j, 64 slots = 40 real + 24 zero):
- Table tab[16g+ch, 1+loc, j] = Z[g*6250+loc, ch+16j]  (g = owning core).
- Edges (partitioned by target shard) are assigned to the 16-partition
  GROUP of their source node; each Q7 core gathers its group's edge
  stream via ap_gather (per-group index lists run in parallel).
- Per-(core,group) targets are rank-sorted by in-count; shared width
  profile W[p] = max over the 64 (core,group) profiles keeps the
  program SPMD-identical; DVE window reduces produce per-group partial
  sums `red` in rank order.
- An ap_gather "align" pass permutes red back to natural target order;
  a PE one-hot matmul sums the 8 group blocks + the self-loop term
  (per-core one-hot E_c selects the local table block).
- Drains scale by dinv^2 (hop1) / dinv (hop2) + bias; softmax via PE
  transposes back to node-major.

Collectives: AllGather of each core's [16, 25088] f16 z-block between
hops; one strided DMA rebuilds the table.
"""
import numpy as np

import concourse.bacc as bacc
import concourse.mybir as mybir
import concourse.tile as tile
from concourse.bass_utils import run_bass_kernel_spmd

F16 = mybir.dt.float16
F32 = mybir.dt.float32
I16 = mybir.dt.int16

N = 50000
F_IN = 100
C = 40
CORES = 8
NPC = 6250
GROUPS = 8
TABN = 6273               # table elems per group (slot 0 = zeros, 49*128 + 1)
SEG = 1792                # max gather lanes per instruction
ACH = 384                 # targets per align-gather chunk (3 blocks of 128)
NBLK = 49                 # ceil(NPC / 128) target blocks
BLKT = 128                # targets per combine block
BLKF = BLKT * 4           # 512 free elems per block
CCFULL = NBLK * BLKF      # 25088 cc row width, element-major like the
                          # table: cc[ch, e*4 + j] = Z slot (ch+16j, e)
TABP = NBLK * BLKT        # 6272 padded table locs

LAST_RESULTS = None


def wrap16(flat):
    n = flat.shape[0]
    assert n % 16 == 0
    return flat.reshape(n // 16, 16).T.astype(np.int16)


def build_schedule(edge_index):
    row = np.asarray(edge_index[0], dtype=np.int64)
    col = np.asarray(edge_index[1], dtype=np.int64)
    deg = np.bincount(col, minlength=N).astype(np.float64) + 1.0
    dinv = 1.0 / np.sqrt(deg)

    src_of = [[None] * GROUPS for _ in range(CORES)]
    cnt = np.zeros((CORES, GROUPS, NPC), np.int32)
    for c in range(CORES):
        lo = c * NPC
        m = (col >= lo) & (col < lo + NPC)
        s_all, t_all = row[m], col[m] - lo
        g_all, sloc_all = s_all // NPC, s_all % NPC
        for g in range(GROUPS):
            gm = g_all == g
            src_of[c][g] = (t_all[gm], sloc_all[gm])
            np.add.at(cnt[c, g], t_all[gm], 1)

    sorted_cnt = -np.sort(-cnt.reshape(CORES * GROUPS, NPC), axis=1)
    W = sorted_cnt.max(axis=0)
    nslots = int((W > 0).sum())
    W = W[:nslots].astype(np.int64)
    widths, n_counts = [], []
    p = 0
    while p < nslots:
        w = int(W[p])
        q = p
        while q < nslots and W[q] == w:
            q += 1
        widths.append(w)
        n_counts.append(q - p)
        p = q
    bucket_off = {}
    lane = 0
    slot_total = 0
    for w, nw in zip(widths, n_counts):
        bucket_off[w] = (lane, slot_total)
        lane += nw * w
        slot_total += nw
    L, R = lane, slot_total + 1
    slot_lane_off = np.zeros(nslots + 1, np.int64)
    np.cumsum(W, out=slot_lane_off[1:])

    segs = []
    cur = dict(lanes=0, pieces=[])
    for w, nw in zip(widths, n_counts):
        _, off_slot = bucket_off[w]
        left, red0 = nw, 1 + off_slot
        while left > 0:
            room = (SEG - cur["lanes"]) // w
            if room == 0:
                segs.append(cur)
                cur = dict(lanes=0, pieces=[])
                room = SEG // w
            take = min(left, room)
            cur["pieces"].append((cur["lanes"], take, w, red0))
            cur["lanes"] += take * w
            red0 += take
            left -= take
    if cur["pieces"]:
        segs.append(cur)
    for s in segs:
        s["pad_lanes"] = (-s["lanes"]) % 16

    pos = np.zeros((CORES, GROUPS, NPC), np.int32)
    gidx = [[None] * GROUPS for _ in range(CORES)]
    for c in range(CORES):
        for g in range(GROUPS):
            t_a, s_a = src_of[c][g]
            order = np.argsort(t_a, kind="stable")
            s_s = s_a[order]
            cnts = cnt[c, g]
            starts = np.zeros(NPC + 1, np.int64)
            np.cumsum(cnts, out=starts[1:])
            flat = np.zeros(L, np.int16)
            rank_t = np.argsort(-cnts, kind="stable")[:nslots]
            for p_ in range(nslots):
                t = rank_t[p_]
                k = cnts[t]
                if k == 0:
                    continue
                a = starts[t]
                lo = slot_lane_off[p_]
                flat[lo: lo + k] = 1 + s_s[a:a + k]
                pos[c, g, t] = 1 + p_
            gidx[c][g] = flat

    seg_idx = []
    for c in range(CORES):
        off, parts = 0, []
        for s in segs:
            ln = s["lanes"] + s["pad_lanes"]
            arr = np.zeros((128, ln // 16), np.int16)
            for g in range(GROUPS):
                fl = np.zeros(ln, np.int16)
                fl[: s["lanes"]] = gidx[c][g][off: off + s["lanes"]]
                arr[16 * g: 16 * (g + 1)] = wrap16(fl)
            parts.append(arr)
            off += s["lanes"]
        seg_idx.append(np.concatenate(parts, axis=1))

    n_chunks = -(-NPC // ACH)
    align_idx = []
    for c in range(CORES):
        parts = []
        for k in range(n_chunks):
            t0 = k * ACH
            tt = np.arange(t0, min(t0 + ACH, NPC))
            arr = np.zeros((128, ACH // 16), np.int16)
            for g in range(GROUPS):
                fl = np.zeros(ACH, np.int16)
                fl[: tt.shape[0]] = pos[c, g, tt]
                arr[16 * g: 16 * (g + 1)] = wrap16(fl)
            parts.append(arr)
        align_idx.append(np.concatenate(parts, axis=1))

    shape = dict(L=L, R=R, segs=segs, n_chunks=n_chunks)
    return shape, seg_idx, align_idx, dinv


def pack_dconst(vals):
    """[NPC] target-wise values -> [16, TABP] replicated drain const."""
    out = np.zeros((16, TABP), np.float16)
    out[:, 0:NPC] = vals.astype(np.float16)[None, :]
    return out


def build_program(shape, sidx_w, aidx_w):
    nc = bacc.Bacc("TRN2", target_bir_lowering=False, debug=False,
                   num_devices=CORES)
    R = shape["R"]
    segs = shape["segs"]
    n_chunks = shape["n_chunks"]

    xT = nc.dram_tensor("xT", [F_IN, NPC], F16, kind="ExternalInput")
    W64 = nc.dram_tensor("W64", [F_IN, 64], F16, kind="ExternalInput")
    bw = nc.dram_tensor("bw", [16, 4], F32, kind="ExternalInput")
    dvh = nc.dram_tensor("dvh", [16, NPC], F16, kind="ExternalInput")
    dc2 = nc.dram_tensor("dc2", [16, TABP], F16, kind="ExternalInput")
    dc1 = nc.dram_tensor("dc1", [16, TABP], F16, kind="ExternalInput")
    gseg = nc.dram_tensor("gseg", [128, sidx_w], I16, kind="ExternalInput")
    aseg = nc.dram_tensor("aseg", [128, aidx_w], I16, kind="ExternalInput")
    ec = nc.dram_tensor("ec", [128, 16], F16, kind="ExternalInput")
    cm = nc.dram_tensor("cm", [128, 16], F16, kind="ExternalInput")
    ident = nc.dram_tensor("ident", [128, 128], F16, kind="ExternalInput")
    out = nc.dram_tensor("out", [NPC, C], F32, kind="ExternalOutput")

    cc1 = nc.dram_tensor("cc1", [16, CCFULL], F16)
    cc2 = nc.dram_tensor("cc2", [16, CCFULL], F16)
    zg1 = nc.dram_tensor("zg1", [128, CCFULL], F16, addr_space="Shared")
    zg2 = nc.dram_tensor("zg2", [128, CCFULL], F16, addr_space="Shared")

    with tile.TileContext(nc) as tc:
        with tc.tile_pool(name="const", bufs=1) as cpool, \
             tc.tile_pool(name="big", bufs=1) as big, \
             tc.tile_pool(name="msgs", bufs=2) as mpool, \
             tc.tile_pool(name="alg", bufs=2) as apool, \
             tc.tile_pool(name="ps", bufs=2, space="PSUM") as ps, \
             tc.tile_pool(name="psc", bufs=4, space="PSUM") as psc:

            W_sb = cpool.tile([F_IN, 64], F16)
            nc.sync.dma_start(out=W_sb[:], in_=W64[:, :])
            bw_sb = cpool.tile([16, 4], F32)
            nc.sync.dma_start(out=bw_sb[:], in_=bw[:, :])
            dvh_sb = cpool.tile([16, NPC], F16)
            nc.sync.dma_start(out=dvh_sb[:], in_=dvh[:, :])
            dc_sb = cpool.tile([16, TABP], F16)
            nc.sync.dma_start(out=dc_sb[:], in_=dc2[:, :])
            gseg_sb = cpool.tile([128, sidx_w], I16)
            nc.sync.dma_start(out=gseg_sb[:], in_=gseg[:, :])
            aseg_sb = cpool.tile([128, aidx_w], I16)
            nc.sync.dma_start(out=aseg_sb[:], in_=aseg[:, :])
            ec_sb = cpool.tile([128, 16], F16)
            nc.sync.dma_start(out=ec_sb[:], in_=ec[:, :])
            cm_sb = cpool.tile([128, 16], F16)
            nc.sync.dma_start(out=cm_sb[:], in_=cm[:, :])
            id_sb = cpool.tile([128, 128], F16)
            nc.sync.dma_start(out=id_sb[:], in_=ident[:, :])
            xT_sb = cpool.tile([F_IN, NPC], F16)
            nc.sync.dma_start(out=xT_sb[:], in_=xT[:, :])

            tab = big.tile([128, TABN, 4], F16)
            nc.vector.memset(tab[:], 0.0)
            red = big.tile([128, R, 4], F16)
            nc.vector.memset(red[:, 0:1, :], 0.0)

            # ---- head: cc1 blocks = dinv * (x @ W), (t, j) element-major;
            # one matmul per j-plane keeps every op on partitions 0:16 ----
            for b in range(NBLK):
                n0 = b * BLKT
                nt = min(BLKT, NPC - n0)
                st = mpool.tile([16, BLKF], F16, tag="dst")
                if nt < BLKT:
                    nc.vector.memset(st[:], 0.0)
                stv = st[:].rearrange("ch (t j) -> ch t j", j=4)
                for j in range(4):
                    bank = ps.tile([16, BLKT], F32, tag="hbank")
                    nc.tensor.matmul(out=bank[:, 0:nt],
                                     lhsT=W_sb[:, 16 * j:16 * (j + 1)],
                                     rhs=xT_sb[:, n0:n0 + nt],
                                     start=True, stop=True)
                    nc.vector.tensor_tensor(
                        out=stv[:, 0:nt, j], in0=bank[:, 0:nt],
                        in1=dvh_sb[:, n0:n0 + nt],
                        op=mybir.AluOpType.mult)
                nc.sync.dma_start(
                    out=cc1[:, b * BLKF:(b + 1) * BLKF], in_=st[:])

            def allgather(cc, zg):
                nc.gpsimd.collective_compute(
                    "AllGather", mybir.AluOpType.bypass,
                    replica_groups=[list(range(CORES))],
                    ins=[cc[:, :].opt()], outs=[zg[:, :].opt()])
                nc.sync.dma_start(out=tab[:, 1:TABN, :], in_=zg[:, :])

            def hop(dc_sb, tag, finish):
                soff = 0
                for si, s in enumerate(segs):
                    ln = s["lanes"] + s["pad_lanes"]
                    msgs = mpool.tile([128, SEG + 16, 4], F16, tag="m")
                    nc.gpsimd.ap_gather(
                        out_ap=msgs[:, 0:ln, :], in_ap=tab[:],
                        idxs_ap=gseg_sb[:, soff:soff + ln // 16],
                        channels=128, num_elems=TABN, d=4, num_idxs=ln)
                    with nc.allow_low_precision(reason="f16 segsum"):
                        for (lo, nsl, w, r0) in s["pieces"]:
                            nc.vector.tensor_reduce(
                                out=red[:, r0:r0 + nsl, :],
                                in_=msgs[:, lo:lo + nsl * w, :].rearrange(
                                    "p (n w) d -> p n d w", w=w),
                                axis=mybir.AxisListType.X,
                                op=mybir.AluOpType.add)
                    soff += ln // 16
                for k in range(n_chunks):
                    alg = apool.tile([128, ACH, 4], F16, tag="a")
                    nc.gpsimd.ap_gather(
                        out_ap=alg[:], in_ap=red[:],
                        idxs_ap=aseg_sb[:, k * (ACH // 16):
                                        (k + 1) * (ACH // 16)],
                        channels=128, num_elems=R, d=4, num_idxs=ACH)
                    for bb in range(ACH // BLKT):
                        b = k * (ACH // BLKT) + bb
                        if b >= NBLK:
                            break
                        n0 = b * BLKT
                        nt = min(BLKT, NPC - n0)
                        bank = psc.tile([16, BLKF], F32, tag="cbank")
                        nc.tensor.matmul(
                            out=bank[:, 0:nt * 4], lhsT=cm_sb[:],
                            rhs=alg[:, bb * BLKT:bb * BLKT + nt, :],
                            start=True, stop=False)
                        nc.tensor.matmul(
                            out=bank[:, 0:nt * 4], lhsT=ec_sb[:],
                            rhs=tab[:, 1 + n0:1 + n0 + nt, :],
                            start=False, stop=True)
                        finish(b, nt, bank, dc_sb)

            def drain1(b, nt, bank, dc_sb):
                n0 = b * BLKT
                st = mpool.tile([16, BLKF], F16, tag="dst")
                if nt < BLKT:
                    nc.vector.memset(st[:], 0.0)
                nc.vector.tensor_tensor(
                    out=st[:].rearrange("ch (t j) -> ch t j", j=4)[
                        :, 0:nt, :],
                    in0=bank[:, 0:nt * 4].rearrange(
                        "ch (t j) -> ch t j", j=4),
                    in1=dc_sb[:, n0:n0 + nt].unsqueeze(2).to_broadcast(
                        [16, nt, 4]),
                    op=mybir.AluOpType.mult)
                nc.sync.dma_start(
                    out=cc2[:, b * BLKF:(b + 1) * BLKF], in_=st[:])

            nm = big.tile([128, NBLK, 48], F16)

            def drain2(b, nt, bank, dc_sb):
                n0 = b * BLKT
                st = mpool.tile([16, BLKF], F16, tag="dst")
                if nt < BLKT:
                    nc.vector.memset(st[:], 0.0)
                stv = st[:].rearrange("ch (t j) -> ch t j", j=4)
                nc.vector.tensor_tensor(
                    out=stv[:, 0:nt, :],
                    in0=bank[:, 0:nt * 4].rearrange(
                        "ch (t j) -> ch t j", j=4),
                    in1=dc_sb[:, n0:n0 + nt].unsqueeze(2).to_broadcast(
                        [16, nt, 4]),
                    op=mybir.AluOpType.mult)
                nc.vector.tensor_tensor(
                    out=stv[:, 0:nt, :], in0=stv[:, 0:nt, :],
                    in1=bw_sb[:, :].unsqueeze(1).to_broadcast([16, nt, 4]),
                    op=mybir.AluOpType.add)
                nbank = ps.tile([128, 48], F16, tag="tbank")
                for j in range(3):
                    nc.tensor.transpose(
                        out=nbank[:, 16 * j:16 * (j + 1)],
                        in_=stv[:, :, j],
                        identity=id_sb[0:16, 0:16])
                nc.vector.tensor_copy(out=nm[:, b, :], in_=nbank[:])

            allgather(cc1, zg1)
            hop(dc_sb, "h1", drain1)
            nc.sync.dma_start(out=dc_sb[:], in_=dc1[:, :])
            allgather(cc2, zg2)
            hop(dc_sb, "h2", drain2)

            # ---- softmax (node-major tiles built by drain2) ----
            mx = big.tile([128, NBLK], F32)
            nc.vector.tensor_reduce(out=mx[:], in_=nm[:, :, 0:C],
                                    axis=mybir.AxisListType.X,
                                    op=mybir.AluOpType.max)
            sh = big.tile([128, NBLK, C], F16)
            nc.vector.tensor_tensor(
                out=sh[:], in0=nm[:, :, 0:C],
                in1=mx[:].unsqueeze(2).to_broadcast([128, NBLK, C]),
                op=mybir.AluOpType.subtract)
            ex = big.tile([128, NBLK, C], F16)
            nc.scalar.activation(out=ex[:], in_=sh[:],
                                 func=mybir.ActivationFunctionType.Exp)
            sm = big.tile([128, NBLK], F32)
            nc.vector.tensor_reduce(out=sm[:], in_=ex[:],
                                    axis=mybir.AxisListType.X,
                                    op=mybir.AluOpType.add)
            lsm = big.tile([128, NBLK], F32)
            nc.scalar.activation(out=lsm[:], in_=sm[:],
                                 func=mybir.ActivationFunctionType.Ln)
            res = big.tile([128, NBLK, C], F32)
            nc.vector.tensor_tensor(
                out=res[:], in0=sh[:],
                in1=lsm[:].unsqueeze(2).to_broadcast([128, NBLK, C]),
                op=mybir.AluOpType.subtract)
            nc.sync.dma_start(
                out=out[0:48 * BLKT, :].rearrange("(b n) c -> n b c", n=BLKT),
                in_=res[:, 0:48, :])
            nc.sync.dma_start(out=out[48 * BLKT:NPC, :],
                              in_=res[0:NPC - 48 * BLKT, 48, :])
    nc.compile()
    return nc


def kernel(x, edge_index, W, b, _trace=False, _sim=False):
    global LAST_RESULTS
    x = np.asarray(x, dtype=np.float32)
    W_ = np.asarray(W, dtype=np.float32)
    b_ = np.asarray(b, dtype=np.float32)
    shape, seg_idx, align_idx, dinv = build_schedule(np.asarray(edge_index))

    W64v = np.zeros((F_IN, 64), np.float16)
    W64v[:, 0:C] = W_.astype(np.float16)
    bwv = np.zeros((16, 4), np.float32)
    for f in range(C):
        bwv[f % 16, f // 16] = b_[f]
    cmv = np.tile(np.eye(16, dtype=np.float16), (8, 1))
    idv = np.eye(128, dtype=np.float16)

    sidx_w = seg_idx[0].shape[1]
    aidx_w = align_idx[0].shape[1]
    nc = build_program(shape, sidx_w, aidx_w)

    in_maps = []
    for c in range(CORES):
        lo = c * NPC
        dl = dinv[lo:lo + NPC]
        ecv = np.zeros((128, 16), np.float16)
        ecv[16 * c:16 * (c + 1)] = np.eye(16, dtype=np.float16)
        in_maps.append({
            "xT": x[lo:lo + NPC, :].T.astype(np.float16),
            "W64": W64v, "bw": bwv,
            "dvh": np.tile(dl.astype(np.float16)[None, :], (16, 1)),
            "dc2": pack_dconst(dl * dl), "dc1": pack_dconst(dl),
            "gseg": seg_idx[c], "aseg": align_idx[c],
            "ec": ecv, "cm": cmv, "ident": idv,
        })

    if _sim:
        import concourse.bass_interp as bass_interp
        sim = bass_interp.MultiCoreSim(nc, CORES)
        for c in range(CORES):
            for k, v in in_maps[c].items():
                sim.cores[c].tensor(k)[:] = v
        sim.simulate()
        outs = [np.array(sim.cores[c].mem_tensor("out"))
                for c in range(CORES)]
        return np.concatenate(outs, axis=0)

    if _trace:
        import ntff_shim  # noqa: F401
    res = run_bass_kernel_spmd(nc, in_maps, core_ids=list(range(CORES)),
                               trace=_trace)
    LAST_RESULTS = res
    return np.concatenate([res.results[c]["out"] for c in range(CORES)],
                          axis=0)


# revision 3
# speedup vs baseline: 1.0258x; 1.0258x over previous
"""2-hop GCN on 8 trn2 cores — GPSIMD ap_gather architecture.

out = log_softmax((D^-.5 (A+I) D^-.5)^2 x W + b)

Device layout (feature slot f = ch + 16j, 64 slots = 40 real + 24 zero):
- Table tab[16g+ch, 1+loc, j] = Z[g*6250+loc, ch+16j]  (g = owning core).
- Edges (partitioned by target shard) are assigned to the 16-partition
  GROUP of their source node; each Q7 core gathers its group's edge
  stream via ap_gather (per-group index lists run in parallel).
- Per-(core,group) targets are rank-sorted by in-count; shared width
  profile W[p] = max over the 64 (core,group) profiles keeps the
  program SPMD-identical; DVE window reduces produce per-group partial
  sums `red` in rank order.
- An ap_gather "align" pass permutes red back to natural target order;
  a PE one-hot matmul sums the 8 group blocks + the self-loop term
  (per-core one-hot E_c selects the local table block).
- Drains scale by dinv^2 (hop1) / dinv (hop2) + bias; softmax via PE
  transposes back to node-major.

Collectives: AllGather of each core's [16, 25088] f16 z-block between
hops; one strided DMA rebuilds the table.
"""
import numpy as np

import concourse.bacc as bacc
import concourse.mybir as mybir
import concourse.tile as tile
from concourse.bass_utils import run_bass_kernel_spmd

F16 = mybir.dt.float16
F32 = mybir.dt.float32
I16 = mybir.dt.int16

N = 50000
F_IN = 100
C = 40
CORES = 8
NPC = 6250
GROUPS = 8
TABN = 6273               # table elems per group (slot 0 = zeros, 49*128 + 1)
SEG = 1792                # max gather lanes per instruction
ACH = 384                 # targets per align-gather chunk (3 blocks of 128)
NBLK = 49                 # ceil(NPC / 128) target blocks
BLKT = 128                # targets per combine block
BLKF = BLKT * 4           # 512 free elems per block
CCFULL = NBLK * BLKF      # 25088 cc row width, element-major like the
                          # table: cc[ch, e*4 + j] = Z slot (ch+16j, e)
TABP = NBLK * BLKT        # 6272 padded table locs

LAST_RESULTS = None


def wrap16(flat):
    n = flat.shape[0]
    assert n % 16 == 0
    return flat.reshape(n // 16, 16).T.astype(np.int16)


def build_schedule(edge_index):
    row = np.asarray(edge_index[0], dtype=np.int64)
    col = np.asarray(edge_index[1], dtype=np.int64)
    deg = np.bincount(col, minlength=N).astype(np.float64) + 1.0
    dinv = 1.0 / np.sqrt(deg)

    src_of = [[None] * GROUPS for _ in range(CORES)]
    cnt = np.zeros((CORES, GROUPS, NPC), np.int32)
    for c in range(CORES):
        lo = c * NPC
        m = (col >= lo) & (col < lo + NPC)
        s_all, t_all = row[m], col[m] - lo
        g_all, sloc_all = s_all // NPC, s_all % NPC
        for g in range(GROUPS):
            gm = g_all == g
            src_of[c][g] = (t_all[gm], sloc_all[gm])
            np.add.at(cnt[c, g], t_all[gm], 1)

    sorted_cnt = -np.sort(-cnt.reshape(CORES * GROUPS, NPC), axis=1)
    W = sorted_cnt.max(axis=0)
    nslots = int((W > 0).sum())
    W = W[:nslots].astype(np.int64)
    widths, n_counts = [], []
    p = 0
    while p < nslots:
        w = int(W[p])
        q = p
        while q < nslots and W[q] == w:
            q += 1
        widths.append(w)
        n_counts.append(q - p)
        p = q
    bucket_off = {}
    lane = 0
    slot_total = 0
    for w, nw in zip(widths, n_counts):
        bucket_off[w] = (lane, slot_total)
        lane += nw * w
        slot_total += nw
    L, R = lane, slot_total + 1
    slot_lane_off = np.zeros(nslots + 1, np.int64)
    np.cumsum(W, out=slot_lane_off[1:])

    segs = []
    cur = dict(lanes=0, pieces=[])
    for w, nw in zip(widths, n_counts):
        _, off_slot = bucket_off[w]
        left, red0 = nw, 1 + off_slot
        while left > 0:
            room = (SEG - cur["lanes"]) // w
            if room == 0:
                segs.append(cur)
                cur = dict(lanes=0, pieces=[])
                room = SEG // w
            take = min(left, room)
            cur["pieces"].append((cur["lanes"], take, w, red0))
            cur["lanes"] += take * w
            red0 += take
            left -= take
    if cur["pieces"]:
        segs.append(cur)
    for s in segs:
        s["pad_lanes"] = (-s["lanes"]) % 16

    pos = np.zeros((CORES, GROUPS, NPC), np.int32)
    gidx = [[None] * GROUPS for _ in range(CORES)]
    for c in range(CORES):
        for g in range(GROUPS):
            t_a, s_a = src_of[c][g]
            order = np.argsort(t_a, kind="stable")
            s_s = s_a[order]
            cnts = cnt[c, g]
            starts = np.zeros(NPC + 1, np.int64)
            np.cumsum(cnts, out=starts[1:])
            flat = np.zeros(L, np.int16)
            rank_t = np.argsort(-cnts, kind="stable")[:nslots]
            for p_ in range(nslots):
                t = rank_t[p_]
                k = cnts[t]
                if k == 0:
                    continue
                a = starts[t]
                lo = slot_lane_off[p_]
                flat[lo: lo + k] = 1 + s_s[a:a + k]
                pos[c, g, t] = 1 + p_
            gidx[c][g] = flat

    seg_idx = []
    for c in range(CORES):
        off, parts = 0, []
        for s in segs:
            ln = s["lanes"] + s["pad_lanes"]
            arr = np.zeros((128, ln // 16), np.int16)
            for g in range(GROUPS):
                fl = np.zeros(ln, np.int16)
                fl[: s["lanes"]] = gidx[c][g][off: off + s["lanes"]]
                arr[16 * g: 16 * (g + 1)] = wrap16(fl)
            parts.append(arr)
            off += s["lanes"]
        seg_idx.append(np.concatenate(parts, axis=1))

    n_chunks = -(-NPC // ACH)
    align_idx = []
    for c in range(CORES):
        parts = []
        for k in range(n_chunks):
            t0 = k * ACH
            tt = np.arange(t0, min(t0 + ACH, NPC))
            arr = np.zeros((128, ACH // 16), np.int16)
            for g in range(GROUPS):
                fl = np.zeros(ACH, np.int16)
                fl[: tt.shape[0]] = pos[c, g, tt]
                arr[16 * g: 16 * (g + 1)] = wrap16(fl)
            parts.append(arr)
        align_idx.append(np.concatenate(parts, axis=1))

    shape = dict(L=L, R=R, segs=segs, n_chunks=n_chunks)
    return shape, seg_idx, align_idx, dinv


def pack_dconst(vals):
    """[NPC] target-wise values -> [16, TABP] replicated drain const."""
    out = np.zeros((16, TABP), np.float16)
    out[:, 0:NPC] = vals.astype(np.float16)[None, :]
    return out


def build_program(shape, sidx_w, aidx_w):
    nc = bacc.Bacc("TRN2", target_bir_lowering=False, debug=False,
                   num_devices=CORES)
    R = shape["R"]
    segs = shape["segs"]
    n_chunks = shape["n_chunks"]

    xT = nc.dram_tensor("xT", [F_IN, NPC], F16, kind="ExternalInput")
    W64 = nc.dram_tensor("W64", [F_IN, 64], F16, kind="ExternalInput")
    bw = nc.dram_tensor("bw", [16, 4], F32, kind="ExternalInput")
    dvh = nc.dram_tensor("dvh", [16, NPC], F16, kind="ExternalInput")
    dc2 = nc.dram_tensor("dc2", [16, TABP], F16, kind="ExternalInput")
    dc1 = nc.dram_tensor("dc1", [16, TABP], F16, kind="ExternalInput")
    gseg = nc.dram_tensor("gseg", [128, sidx_w], I16, kind="ExternalInput")
    aseg = nc.dram_tensor("aseg", [128, aidx_w], I16, kind="ExternalInput")
    ec = nc.dram_tensor("ec", [128, 16], F16, kind="ExternalInput")
    cm = nc.dram_tensor("cm", [128, 16], F16, kind="ExternalInput")
    ident = nc.dram_tensor("ident", [128, 128], F16, kind="ExternalInput")
    out = nc.dram_tensor("out", [NPC, C], F32, kind="ExternalOutput")

    cc1 = nc.dram_tensor("cc1", [16, CCFULL], F16)
    cc2 = nc.dram_tensor("cc2", [16, CCFULL], F16)
    zg1 = nc.dram_tensor("zg1", [128, CCFULL], F16, addr_space="Shared")
    zg2 = nc.dram_tensor("zg2", [128, CCFULL], F16, addr_space="Shared")

    with tile.TileContext(nc) as tc:
        with tc.tile_pool(name="const", bufs=1) as cpool, \
             tc.tile_pool(name="big", bufs=1) as big, \
             tc.tile_pool(name="msgs", bufs=2) as mpool, \
             tc.tile_pool(name="alg", bufs=3) as apool, \
             tc.tile_pool(name="ps", bufs=2, space="PSUM") as ps, \
             tc.tile_pool(name="psc", bufs=4, space="PSUM") as psc:

            W_sb = cpool.tile([F_IN, 64], F16)
            nc.sync.dma_start(out=W_sb[:], in_=W64[:, :])
            bw_sb = cpool.tile([16, 4], F32)
            nc.sync.dma_start(out=bw_sb[:], in_=bw[:, :])
            dvh_sb = cpool.tile([16, NPC], F16)
            nc.sync.dma_start(out=dvh_sb[:], in_=dvh[:, :])
            dc_sb = cpool.tile([16, TABP], F16)
            nc.sync.dma_start(out=dc_sb[:], in_=dc2[:, :])
            gseg_sb = cpool.tile([128, sidx_w], I16)
            nc.sync.dma_start(out=gseg_sb[:], in_=gseg[:, :])
            aseg_sb = cpool.tile([128, aidx_w], I16)
            nc.sync.dma_start(out=aseg_sb[:], in_=aseg[:, :])
            ec_sb = cpool.tile([128, 16], F16)
            nc.sync.dma_start(out=ec_sb[:], in_=ec[:, :])
            cm_sb = cpool.tile([128, 16], F16)
            nc.sync.dma_start(out=cm_sb[:], in_=cm[:, :])
            id_sb = cpool.tile([128, 128], F16)
            nc.sync.dma_start(out=id_sb[:], in_=ident[:, :])
            xT_sb = cpool.tile([F_IN, NPC], F16)
            nc.sync.dma_start(out=xT_sb[:], in_=xT[:, :])

            tab = big.tile([128, TABN, 4], F16)
            nc.vector.memset(tab[:], 0.0)
            red = big.tile([128, R, 4], F16)
            nc.vector.memset(red[:, 0:1, :], 0.0)

            # ---- head: cc1 blocks = dinv * (x @ W), (t, j) element-major;
            # one matmul per j-plane keeps every op on partitions 0:16;
            # block PAIRS per instruction to amortize overheads ----
            for bb in range(0, NBLK, 2):
                n0 = bb * BLKT
                nt2 = min(2 * BLKT, NPC - n0)
                wlen = BLKF * ((nt2 + BLKT - 1) // BLKT)
                st = mpool.tile([16, 2 * BLKF], F16, tag="hst")
                if nt2 < 2 * BLKT:
                    nc.vector.memset(st[:], 0.0)
                stv = st[:].rearrange("ch (t j) -> ch t j", j=4)
                for j in range(4):
                    bank = ps.tile([16, 2 * BLKT], F32, tag="hbank")
                    nc.tensor.matmul(out=bank[:, 0:nt2],
                                     lhsT=W_sb[:, 16 * j:16 * (j + 1)],
                                     rhs=xT_sb[:, n0:n0 + nt2],
                                     start=True, stop=True)
                    nc.vector.tensor_tensor(
                        out=stv[:, 0:nt2, j], in0=bank[:, 0:nt2],
                        in1=dvh_sb[:, n0:n0 + nt2],
                        op=mybir.AluOpType.mult)
                nc.sync.dma_start(
                    out=cc1[:, bb * BLKF:bb * BLKF + wlen],
                    in_=st[:, 0:wlen])

            def allgather(cc, zg):
                nc.gpsimd.collective_compute(
                    "AllGather", mybir.AluOpType.bypass,
                    replica_groups=[list(range(CORES))],
                    ins=[cc[:, :].opt()], outs=[zg[:, :].opt()])
                nc.sync.dma_start(out=tab[:, 1:TABN, :], in_=zg[:, :])

            def hop(dc_sb, tag, finish):
                soff = 0
                for si, s in enumerate(segs):
                    ln = s["lanes"] + s["pad_lanes"]
                    msgs = mpool.tile([128, SEG + 16, 4], F16, tag="m")
                    nc.gpsimd.ap_gather(
                        out_ap=msgs[:, 0:ln, :], in_ap=tab[:],
                        idxs_ap=gseg_sb[:, soff:soff + ln // 16],
                        channels=128, num_elems=TABN, d=4, num_idxs=ln)
                    with nc.allow_low_precision(reason="f16 segsum"):
                        for (lo, nsl, w, r0) in s["pieces"]:
                            nc.vector.tensor_reduce(
                                out=red[:, r0:r0 + nsl, :],
                                in_=msgs[:, lo:lo + nsl * w, :].rearrange(
                                    "p (n w) d -> p n d w", w=w),
                                axis=mybir.AxisListType.X,
                                op=mybir.AluOpType.add)
                    soff += ln // 16
                for k in range(n_chunks):
                    alg = apool.tile([128, ACH, 4], F16, tag="a")
                    nc.gpsimd.ap_gather(
                        out_ap=alg[:], in_ap=red[:],
                        idxs_ap=aseg_sb[:, k * (ACH // 16):
                                        (k + 1) * (ACH // 16)],
                        channels=128, num_elems=R, d=4, num_idxs=ACH)
                    for bb in range(ACH // BLKT):
                        b = k * (ACH // BLKT) + bb
                        if b >= NBLK:
                            break
                        n0 = b * BLKT
                        nt = min(BLKT, NPC - n0)
                        bank = psc.tile([16, BLKF], F32, tag="cbank")
                        nc.tensor.matmul(
                            out=bank[:, 0:nt * 4], lhsT=cm_sb[:],
                            rhs=alg[:, bb * BLKT:bb * BLKT + nt, :],
                            start=True, stop=False)
                        nc.tensor.matmul(
                            out=bank[:, 0:nt * 4], lhsT=ec_sb[:],
                            rhs=tab[:, 1 + n0:1 + n0 + nt, :],
                            start=False, stop=True)
                        finish(b, nt, bank, dc_sb)

            def drain1(b, nt, bank, dc_sb):
                n0 = b * BLKT
                st = mpool.tile([16, BLKF], F16, tag="dst")
                if nt < BLKT:
                    nc.vector.memset(st[:], 0.0)
                nc.vector.tensor_tensor(
                    out=st[:].rearrange("ch (t j) -> ch t j", j=4)[
                        :, 0:nt, :],
                    in0=bank[:, 0:nt * 4].rearrange(
                        "ch (t j) -> ch t j", j=4),
                    in1=dc_sb[:, n0:n0 + nt].unsqueeze(2).to_broadcast(
                        [16, nt, 4]),
                    op=mybir.AluOpType.mult)
                nc.sync.dma_start(
                    out=cc2[:, b * BLKF:(b + 1) * BLKF], in_=st[:])

            nm = big.tile([128, NBLK, 48], F16)

            def drain2(b, nt, bank, dc_sb):
                n0 = b * BLKT
                st = mpool.tile([16, BLKF], F16, tag="dst")
                if nt < BLKT:
                    nc.vector.memset(st[:], 0.0)
                stv = st[:].rearrange("ch (t j) -> ch t j", j=4)
                nc.vector.tensor_tensor(
                    out=stv[:, 0:nt, :],
                    in0=bank[:, 0:nt * 4].rearrange(
                        "ch (t j) -> ch t j", j=4),
                    in1=dc_sb[:, n0:n0 + nt].unsqueeze(2).to_broadcast(
                        [16, nt, 4]),
                    op=mybir.AluOpType.mult)
                nc.vector.tensor_tensor(
                    out=stv[:, 0:nt, :], in0=stv[:, 0:nt, :],
                    in1=bw_sb[:, :].unsqueeze(1).to_broadcast([16, nt, 4]),
                    op=mybir.AluOpType.add)
                nbank = ps.tile([128, 48], F16, tag="tbank")
                for j in range(3):
                    nc.tensor.transpose(
                        out=nbank[:, 16 * j:16 * (j + 1)],
                        in_=stv[:, :, j],
                        identity=id_sb[0:16, 0:16])
                nc.vector.tensor_copy(out=nm[:, b, :], in_=nbank[:])

            allgather(cc1, zg1)
            hop(dc_sb, "h1", drain1)
            nc.sync.dma_start(out=dc_sb[:], in_=dc1[:, :])
            allgather(cc2, zg2)
            hop(dc_sb, "h2", drain2)

            # ---- softmax (node-major tiles built by drain2) ----
            mx = big.tile([128, NBLK], F32)
            nc.vector.tensor_reduce(out=mx[:], in_=nm[:, :, 0:C],
                                    axis=mybir.AxisListType.X,
                                    op=mybir.AluOpType.max)
            sh = big.tile([128, NBLK, C], F16)
            nc.vector.tensor_tensor(
                out=sh[:], in0=nm[:, :, 0:C],
                in1=mx[:].unsqueeze(2).to_broadcast([128, NBLK, C]),
                op=mybir.AluOpType.subtract)
            ex = big.tile([128, NBLK, C], F16)
            nc.scalar.activation(out=ex[:], in_=sh[:],
                                 func=mybir.ActivationFunctionType.Exp)
            sm = big.tile([128, NBLK], F32)
            nc.vector.tensor_reduce(out=sm[:], in_=ex[:],
                                    axis=mybir.AxisListType.X,
                                    op=mybir.AluOpType.add)
            lsm = big.tile([128, NBLK], F32)
            nc.scalar.activation(out=lsm[:], in_=sm[:],
                                 func=mybir.ActivationFunctionType.Ln)
            res = big.tile([128, NBLK, C], F32)
            nc.vector.tensor_tensor(
                out=res[:], in0=sh[:],
                in1=lsm[:].unsqueeze(2).to_broadcast([128, NBLK, C]),
                op=mybir.AluOpType.subtract)
            nc.sync.dma_start(
                out=out[0:48 * BLKT, :].rearrange("(b n) c -> n b c", n=BLKT),
                in_=res[:, 0:48, :])
            nc.sync.dma_start(out=out[48 * BLKT:NPC, :],
                              in_=res[0:NPC - 48 * BLKT, 48, :])
    nc.compile()
    return nc


def kernel(x, edge_index, W, b, _trace=False, _sim=False):
    global LAST_RESULTS
    x = np.asarray(x, dtype=np.float32)
    W_ = np.asarray(W, dtype=np.float32)
    b_ = np.asarray(b, dtype=np.float32)
    shape, seg_idx, align_idx, dinv = build_schedule(np.asarray(edge_index))

    W64v = np.zeros((F_IN, 64), np.float16)
    W64v[:, 0:C] = W_.astype(np.float16)
    bwv = np.zeros((16, 4), np.float32)
    for f in range(C):
        bwv[f % 16, f // 16] = b_[f]
    cmv = np.tile(np.eye(16, dtype=np.float16), (8, 1))
    idv = np.eye(128, dtype=np.float16)

    sidx_w = seg_idx[0].shape[1]
    aidx_w = align_idx[0].shape[1]
    nc = build_program(shape, sidx_w, aidx_w)

    in_maps = []
    for c in range(CORES):
        lo = c * NPC
        dl = dinv[lo:lo + NPC]
        ecv = np.zeros((128, 16), np.float16)
        ecv[16 * c:16 * (c + 1)] = np.eye(16, dtype=np.float16)
        in_maps.append({
            "xT": x[lo:lo + NPC, :].T.astype(np.float16),
            "W64": W64v, "bw": bwv,
            "dvh": np.tile(dl.astype(np.float16)[None, :], (16, 1)),
            "dc2": pack_dconst(dl * dl), "dc1": pack_dconst(dl),
            "gseg": seg_idx[c], "aseg": align_idx[c],
            "ec": ecv, "cm": cmv, "ident": idv,
        })

    if _sim:
        import concourse.bass_interp as bass_interp
        sim = bass_interp.MultiCoreSim(nc, CORES)
        for c in range(CORES):
            for k, v in in_maps[c].items():
                sim.cores[c].tensor(k)[:] = v
        sim.simulate()
        outs = [np.array(sim.cores[c].mem_tensor("out"))
                for c in range(CORES)]
        return np.concatenate(outs, axis=0)

    if _trace:
        import ntff_shim  # noqa: F401
    res = run_bass_kernel_spmd(nc, in_maps, core_ids=list(range(CORES)),
                               trace=_trace)
    LAST_RESULTS = res
    return np.concatenate([res.results[c]["out"] for c in range(CORES)],
                          axis=0)


# revision 4
# speedup vs baseline: 1.0401x; 1.0139x over previous
"""2-hop GCN on 8 trn2 cores — GPSIMD ap_gather architecture.

out = log_softmax((D^-.5 (A+I) D^-.5)^2 x W + b)

Device layout (feature slot f = ch + 16j, 64 slots = 40 real + 24 zero):
- Table tab[16g+ch, 1+loc, j] = Z[g*6250+loc, ch+16j]  (g = owning core).
- Edges (partitioned by target shard) are assigned to the 16-partition
  GROUP of their source node; each Q7 core gathers its group's edge
  stream via ap_gather (per-group index lists run in parallel).
- Per-(core,group) targets are rank-sorted by in-count; shared width
  profile W[p] = max over the 64 (core,group) profiles keeps the
  program SPMD-identical; DVE window reduces produce per-group partial
  sums `red` in rank order.
- An ap_gather "align" pass permutes red back to natural target order;
  a PE one-hot matmul sums the 8 group blocks + the self-loop term
  (per-core one-hot E_c selects the local table block).
- Drains scale by dinv^2 (hop1) / dinv (hop2) + bias; softmax via PE
  transposes back to node-major.

Collectives: AllGather of each core's [16, 25088] f16 z-block between
hops; one strided DMA rebuilds the table.
"""
import numpy as np

import concourse.bacc as bacc
import concourse.mybir as mybir
import concourse.tile as tile
from concourse.bass_utils import run_bass_kernel_spmd

F16 = mybir.dt.float16
F32 = mybir.dt.float32
I16 = mybir.dt.int16

N = 50000
F_IN = 100
C = 40
CORES = 8
NPC = 6250
GROUPS = 8
TABN = 6273               # table elems per group (slot 0 = zeros, 49*128 + 1)
SEG = 1792                # max gather lanes per instruction
ACH = 384                 # targets per align-gather chunk (3 blocks of 128)
NBLK = 49                 # ceil(NPC / 128) target blocks
BLKT = 128                # targets per combine block
BLKF = BLKT * 4           # 512 free elems per block
CCFULL = NBLK * BLKF      # 25088 cc row width, element-major like the
                          # table: cc[ch, e*4 + j] = Z slot (ch+16j, e)
TABP = NBLK * BLKT        # 6272 padded table locs

LAST_RESULTS = None


def wrap16(flat):
    n = flat.shape[0]
    assert n % 16 == 0
    return flat.reshape(n // 16, 16).T.astype(np.int16)


def build_schedule(edge_index):
    row = np.asarray(edge_index[0], dtype=np.int64)
    col = np.asarray(edge_index[1], dtype=np.int64)
    deg = np.bincount(col, minlength=N).astype(np.float64) + 1.0
    dinv = 1.0 / np.sqrt(deg)

    src_of = [[None] * GROUPS for _ in range(CORES)]
    cnt = np.zeros((CORES, GROUPS, NPC), np.int32)
    for c in range(CORES):
        lo = c * NPC
        m = (col >= lo) & (col < lo + NPC)
        s_all, t_all = row[m], col[m] - lo
        g_all, sloc_all = s_all // NPC, s_all % NPC
        for g in range(GROUPS):
            gm = g_all == g
            src_of[c][g] = (t_all[gm], sloc_all[gm])
            np.add.at(cnt[c, g], t_all[gm], 1)

    sorted_cnt = -np.sort(-cnt.reshape(CORES * GROUPS, NPC), axis=1)
    W = sorted_cnt.max(axis=0)
    nslots = int((W > 0).sum())
    W = W[:nslots].astype(np.int64)
    widths, n_counts = [], []
    p = 0
    while p < nslots:
        w = int(W[p])
        q = p
        while q < nslots and W[q] == w:
            q += 1
        widths.append(w)
        n_counts.append(q - p)
        p = q
    bucket_off = {}
    lane = 0
    slot_total = 0
    for w, nw in zip(widths, n_counts):
        bucket_off[w] = (lane, slot_total)
        lane += nw * w
        slot_total += nw
    L, R = lane, slot_total + 1
    slot_lane_off = np.zeros(nslots + 1, np.int64)
    np.cumsum(W, out=slot_lane_off[1:])

    segs = []
    cur = dict(lanes=0, pieces=[])
    for w, nw in zip(widths, n_counts):
        _, off_slot = bucket_off[w]
        left, red0 = nw, 1 + off_slot
        while left > 0:
            room = (SEG - cur["lanes"]) // w
            if room == 0:
                segs.append(cur)
                cur = dict(lanes=0, pieces=[])
                room = SEG // w
            take = min(left, room)
            cur["pieces"].append((cur["lanes"], take, w, red0))
            cur["lanes"] += take * w
            red0 += take
            left -= take
    if cur["pieces"]:
        segs.append(cur)
    for s in segs:
        s["pad_lanes"] = (-s["lanes"]) % 16

    pos = np.zeros((CORES, GROUPS, NPC), np.int32)
    gidx = [[None] * GROUPS for _ in range(CORES)]
    for c in range(CORES):
        for g in range(GROUPS):
            t_a, s_a = src_of[c][g]
            order = np.argsort(t_a, kind="stable")
            s_s = s_a[order]
            cnts = cnt[c, g]
            starts = np.zeros(NPC + 1, np.int64)
            np.cumsum(cnts, out=starts[1:])
            flat = np.zeros(L, np.int16)
            rank_t = np.argsort(-cnts, kind="stable")[:nslots]
            for p_ in range(nslots):
                t = rank_t[p_]
                k = cnts[t]
                if k == 0:
                    continue
                a = starts[t]
                lo = slot_lane_off[p_]
                flat[lo: lo + k] = 1 + s_s[a:a + k]
                pos[c, g, t] = 1 + p_
            gidx[c][g] = flat

    seg_idx = []
    for c in range(CORES):
        off, parts = 0, []
        for s in segs:
            ln = s["lanes"] + s["pad_lanes"]
            arr = np.zeros((128, ln // 16), np.int16)
            for g in range(GROUPS):
                fl = np.zeros(ln, np.int16)
                fl[: s["lanes"]] = gidx[c][g][off: off + s["lanes"]]
                arr[16 * g: 16 * (g + 1)] = wrap16(fl)
            parts.append(arr)
            off += s["lanes"]
        seg_idx.append(np.concatenate(parts, axis=1))

    n_chunks = -(-NPC // ACH)
    align_idx = []
    for c in range(CORES):
        parts = []
        for k in range(n_chunks):
            t0 = k * ACH
            tt = np.arange(t0, min(t0 + ACH, NPC))
            arr = np.zeros((128, ACH // 16), np.int16)
            for g in range(GROUPS):
                fl = np.zeros(ACH, np.int16)
                fl[: tt.shape[0]] = pos[c, g, tt]
                arr[16 * g: 16 * (g + 1)] = wrap16(fl)
            parts.append(arr)
        align_idx.append(np.concatenate(parts, axis=1))

    shape = dict(L=L, R=R, segs=segs, n_chunks=n_chunks)
    return shape, seg_idx, align_idx, dinv


def pack_dconst(vals):
    """[NPC] target-wise values -> [16, TABP] replicated drain const."""
    out = np.zeros((16, TABP), np.float16)
    out[:, 0:NPC] = vals.astype(np.float16)[None, :]
    return out


def build_program(shape, sidx_w, aidx_w):
    nc = bacc.Bacc("TRN2", target_bir_lowering=False, debug=False,
                   num_devices=CORES)
    R = shape["R"]
    segs = shape["segs"]
    n_chunks = shape["n_chunks"]

    xT = nc.dram_tensor("xT", [F_IN, NPC], F16, kind="ExternalInput")
    W64 = nc.dram_tensor("W64", [F_IN, 64], F16, kind="ExternalInput")
    bw = nc.dram_tensor("bw", [16, 4], F32, kind="ExternalInput")
    dvh = nc.dram_tensor("dvh", [16, NPC], F16, kind="ExternalInput")
    dc2 = nc.dram_tensor("dc2", [16, TABP], F16, kind="ExternalInput")
    dc1 = nc.dram_tensor("dc1", [16, TABP], F16, kind="ExternalInput")
    gseg = nc.dram_tensor("gseg", [128, sidx_w], I16, kind="ExternalInput")
    aseg = nc.dram_tensor("aseg", [128, aidx_w], I16, kind="ExternalInput")
    ec = nc.dram_tensor("ec", [128, 16], F16, kind="ExternalInput")
    cm = nc.dram_tensor("cm", [128, 16], F16, kind="ExternalInput")
    ident = nc.dram_tensor("ident", [128, 128], F16, kind="ExternalInput")
    out = nc.dram_tensor("out", [NPC, C], F32, kind="ExternalOutput")

    dumA = nc.dram_tensor("dumA", [1, 16], F16)
    dumB = nc.dram_tensor("dumB", [8, 16], F16, addr_space="Shared")
    cc1 = nc.dram_tensor("cc1", [16, CCFULL], F16)
    cc2 = nc.dram_tensor("cc2", [16, CCFULL], F16)
    zg1 = nc.dram_tensor("zg1", [128, CCFULL], F16, addr_space="Shared")
    zg2 = nc.dram_tensor("zg2", [128, CCFULL], F16, addr_space="Shared")

    with tile.TileContext(nc) as tc:
        with tc.tile_pool(name="const", bufs=1) as cpool, \
             tc.tile_pool(name="big", bufs=1) as big, \
             tc.tile_pool(name="msgs", bufs=2) as mpool, \
             tc.tile_pool(name="alg", bufs=3) as apool, \
             tc.tile_pool(name="ps", bufs=2, space="PSUM") as ps, \
             tc.tile_pool(name="psc", bufs=4, space="PSUM") as psc:

            warm = cpool.tile([1, 16], F16)
            nc.vector.memset(warm[:], 0.0)
            nc.sync.dma_start(out=dumA[:, :], in_=warm[:])
            nc.gpsimd.collective_compute(
                "AllGather", mybir.AluOpType.bypass,
                replica_groups=[list(range(CORES))],
                ins=[dumA[:, :].opt()], outs=[dumB[:, :].opt()])
            W_sb = cpool.tile([F_IN, 64], F16)
            nc.sync.dma_start(out=W_sb[:], in_=W64[:, :])
            bw_sb = cpool.tile([16, 4], F32)
            nc.sync.dma_start(out=bw_sb[:], in_=bw[:, :])
            dvh_sb = cpool.tile([16, NPC], F16)
            nc.sync.dma_start(out=dvh_sb[:], in_=dvh[:, :])
            dc_sb = cpool.tile([16, TABP], F16)
            nc.sync.dma_start(out=dc_sb[:], in_=dc2[:, :])
            gseg_sb = cpool.tile([128, sidx_w], I16)
            nc.sync.dma_start(out=gseg_sb[:], in_=gseg[:, :])
            aseg_sb = cpool.tile([128, aidx_w], I16)
            nc.sync.dma_start(out=aseg_sb[:], in_=aseg[:, :])
            ec_sb = cpool.tile([128, 16], F16)
            nc.sync.dma_start(out=ec_sb[:], in_=ec[:, :])
            cm_sb = cpool.tile([128, 16], F16)
            nc.sync.dma_start(out=cm_sb[:], in_=cm[:, :])
            id_sb = cpool.tile([128, 128], F16)
            nc.sync.dma_start(out=id_sb[:], in_=ident[:, :])
            xT_sb = cpool.tile([F_IN, NPC], F16)
            nc.sync.dma_start(out=xT_sb[:], in_=xT[:, :])

            tab = big.tile([128, TABN, 4], F16)
            nc.vector.memset(tab[:], 0.0)
            red = big.tile([128, R, 4], F16)
            nc.vector.memset(red[:, 0:1, :], 0.0)

            # ---- head: cc1 blocks = dinv * (x @ W), (t, j) element-major;
            # one matmul per j-plane keeps every op on partitions 0:16;
            # block PAIRS per instruction to amortize overheads ----
            for bb in range(0, NBLK, 2):
                n0 = bb * BLKT
                nt2 = min(2 * BLKT, NPC - n0)
                wlen = BLKF * ((nt2 + BLKT - 1) // BLKT)
                st = mpool.tile([16, 2 * BLKF], F16, tag="hst")
                if nt2 < 2 * BLKT:
                    nc.vector.memset(st[:], 0.0)
                stv = st[:].rearrange("ch (t j) -> ch t j", j=4)
                for j in range(4):
                    bank = ps.tile([16, 2 * BLKT], F32, tag="hbank")
                    nc.tensor.matmul(out=bank[:, 0:nt2],
                                     lhsT=W_sb[:, 16 * j:16 * (j + 1)],
                                     rhs=xT_sb[:, n0:n0 + nt2],
                                     start=True, stop=True)
                    nc.vector.tensor_tensor(
                        out=stv[:, 0:nt2, j], in0=bank[:, 0:nt2],
                        in1=dvh_sb[:, n0:n0 + nt2],
                        op=mybir.AluOpType.mult)
                nc.sync.dma_start(
                    out=cc1[:, bb * BLKF:bb * BLKF + wlen],
                    in_=st[:, 0:wlen])

            def allgather(cc, zg):
                nc.gpsimd.collective_compute(
                    "AllGather", mybir.AluOpType.bypass,
                    replica_groups=[list(range(CORES))],
                    ins=[cc[:, :].opt()], outs=[zg[:, :].opt()])
                half = CCFULL // 2
                nc.sync.dma_start(
                    out=tab[:, 1:1 + TABP // 2, :],
                    in_=zg[:, 0:half])
                nc.scalar.dma_start(
                    out=tab[:, 1 + TABP // 2:TABN, :],
                    in_=zg[:, half:CCFULL])

            def hop(dc_sb, tag, finish):
                soff = 0
                for si, s in enumerate(segs):
                    ln = s["lanes"] + s["pad_lanes"]
                    msgs = mpool.tile([128, SEG + 16, 4], F16, tag="m")
                    nc.gpsimd.ap_gather(
                        out_ap=msgs[:, 0:ln, :], in_ap=tab[:],
                        idxs_ap=gseg_sb[:, soff:soff + ln // 16],
                        channels=128, num_elems=TABN, d=4, num_idxs=ln)
                    with nc.allow_low_precision(reason="f16 segsum"):
                        for (lo, nsl, w, r0) in s["pieces"]:
                            nc.vector.tensor_reduce(
                                out=red[:, r0:r0 + nsl, :],
                                in_=msgs[:, lo:lo + nsl * w, :].rearrange(
                                    "p (n w) d -> p n d w", w=w),
                                axis=mybir.AxisListType.X,
                                op=mybir.AluOpType.add)
                    soff += ln // 16
                for k in range(n_chunks):
                    alg = apool.tile([128, ACH, 4], F16, tag="a")
                    nc.gpsimd.ap_gather(
                        out_ap=alg[:], in_ap=red[:],
                        idxs_ap=aseg_sb[:, k * (ACH // 16):
                                        (k + 1) * (ACH // 16)],
                        channels=128, num_elems=R, d=4, num_idxs=ACH)
                    for bb in range(ACH // BLKT):
                        b = k * (ACH // BLKT) + bb
                        if b >= NBLK:
                            break
                        n0 = b * BLKT
                        nt = min(BLKT, NPC - n0)
                        bank = psc.tile([16, BLKF], F32, tag="cbank")
                        nc.tensor.matmul(
                            out=bank[:, 0:nt * 4], lhsT=cm_sb[:],
                            rhs=alg[:, bb * BLKT:bb * BLKT + nt, :],
                            start=True, stop=False)
                        nc.tensor.matmul(
                            out=bank[:, 0:nt * 4], lhsT=ec_sb[:],
                            rhs=tab[:, 1 + n0:1 + n0 + nt, :],
                            start=False, stop=True)
                        finish(b, nt, bank, dc_sb)

            def drain1(b, nt, bank, dc_sb):
                n0 = b * BLKT
                st = mpool.tile([16, BLKF], F16, tag="dst")
                if nt < BLKT:
                    nc.vector.memset(st[:], 0.0)
                nc.vector.tensor_tensor(
                    out=st[:].rearrange("ch (t j) -> ch t j", j=4)[
                        :, 0:nt, :],
                    in0=bank[:, 0:nt * 4].rearrange(
                        "ch (t j) -> ch t j", j=4),
                    in1=dc_sb[:, n0:n0 + nt].unsqueeze(2).to_broadcast(
                        [16, nt, 4]),
                    op=mybir.AluOpType.mult)
                nc.sync.dma_start(
                    out=cc2[:, b * BLKF:(b + 1) * BLKF], in_=st[:])

            nm = big.tile([128, NBLK, 48], F16)

            def drain2(b, nt, bank, dc_sb):
                n0 = b * BLKT
                st = mpool.tile([16, BLKF], F16, tag="dst")
                if nt < BLKT:
                    nc.vector.memset(st[:], 0.0)
                stv = st[:].rearrange("ch (t j) -> ch t j", j=4)
                nc.vector.tensor_tensor(
                    out=stv[:, 0:nt, :],
                    in0=bank[:, 0:nt * 4].rearrange(
                        "ch (t j) -> ch t j", j=4),
                    in1=dc_sb[:, n0:n0 + nt].unsqueeze(2).to_broadcast(
                        [16, nt, 4]),
                    op=mybir.AluOpType.mult)
                nc.vector.tensor_tensor(
                    out=stv[:, 0:nt, :], in0=stv[:, 0:nt, :],
                    in1=bw_sb[:, :].unsqueeze(1).to_broadcast([16, nt, 4]),
                    op=mybir.AluOpType.add)
                nbank = ps.tile([128, 48], F16, tag="tbank")
                for j in range(3):
                    nc.tensor.transpose(
                        out=nbank[:, 16 * j:16 * (j + 1)],
                        in_=stv[:, :, j],
                        identity=id_sb[0:16, 0:16])
                nc.vector.tensor_copy(out=nm[:, b, :], in_=nbank[:])

            allgather(cc1, zg1)
            hop(dc_sb, "h1", drain1)
            nc.sync.dma_start(out=dc_sb[:], in_=dc1[:, :])
            allgather(cc2, zg2)
            hop(dc_sb, "h2", drain2)

            # ---- softmax (node-major tiles built by drain2) ----
            mx = big.tile([128, NBLK], F32)
            nc.vector.tensor_reduce(out=mx[:], in_=nm[:, :, 0:C],
                                    axis=mybir.AxisListType.X,
                                    op=mybir.AluOpType.max)
            sh = big.tile([128, NBLK, C], F16)
            nc.vector.tensor_tensor(
                out=sh[:], in0=nm[:, :, 0:C],
                in1=mx[:].unsqueeze(2).to_broadcast([128, NBLK, C]),
                op=mybir.AluOpType.subtract)
            ex = big.tile([128, NBLK, C], F16)
            nc.scalar.activation(out=ex[:], in_=sh[:],
                                 func=mybir.ActivationFunctionType.Exp)
            sm = big.tile([128, NBLK], F32)
            nc.vector.tensor_reduce(out=sm[:], in_=ex[:],
                                    axis=mybir.AxisListType.X,
                                    op=mybir.AluOpType.add)
            lsm = big.tile([128, NBLK], F32)
            nc.scalar.activation(out=lsm[:], in_=sm[:],
                                 func=mybir.ActivationFunctionType.Ln)
            res = big.tile([128, NBLK, C], F32)
            nc.vector.tensor_tensor(
                out=res[:], in0=sh[:],
                in1=lsm[:].unsqueeze(2).to_broadcast([128, NBLK, C]),
                op=mybir.AluOpType.subtract)
            nc.sync.dma_start(
                out=out[0:48 * BLKT, :].rearrange("(b n) c -> n b c", n=BLKT),
                in_=res[:, 0:48, :])
            nc.sync.dma_start(out=out[48 * BLKT:NPC, :],
                              in_=res[0:NPC - 48 * BLKT, 48, :])
    nc.compile()
    return nc


def kernel(x, edge_index, W, b, _trace=False, _sim=False):
    global LAST_RESULTS
    x = np.asarray(x, dtype=np.float32)
    W_ = np.asarray(W, dtype=np.float32)
    b_ = np.asarray(b, dtype=np.float32)
    shape, seg_idx, align_idx, dinv = build_schedule(np.asarray(edge_index))

    W64v = np.zeros((F_IN, 64), np.float16)
    W64v[:, 0:C] = W_.astype(np.float16)
    bwv = np.zeros((16, 4), np.float32)
    for f in range(C):
        bwv[f % 16, f // 16] = b_[f]
    cmv = np.tile(np.eye(16, dtype=np.float16), (8, 1))
    idv = np.eye(128, dtype=np.float16)

    sidx_w = seg_idx[0].shape[1]
    aidx_w = align_idx[0].shape[1]
    nc = build_program(shape, sidx_w, aidx_w)

    in_maps = []
    for c in range(CORES):
        lo = c * NPC
        dl = dinv[lo:lo + NPC]
        ecv = np.zeros((128, 16), np.float16)
        ecv[16 * c:16 * (c + 1)] = np.eye(16, dtype=np.float16)
        in_maps.append({
            "xT": x[lo:lo + NPC, :].T.astype(np.float16),
            "W64": W64v, "bw": bwv,
            "dvh": np.tile(dl.astype(np.float16)[None, :], (16, 1)),
            "dc2": pack_dconst(dl * dl), "dc1": pack_dconst(dl),
            "gseg": seg_idx[c], "aseg": align_idx[c],
            "ec": ecv, "cm": cmv, "ident": idv,
        })

    if _sim:
        import concourse.bass_interp as bass_interp
        sim = bass_interp.MultiCoreSim(nc, CORES)
        for c in range(CORES):
            for k, v in in_maps[c].items():
                sim.cores[c].tensor(k)[:] = v
        sim.simulate()
        outs = [np.array(sim.cores[c].mem_tensor("out"))
                for c in range(CORES)]
        return np.concatenate(outs, axis=0)

    if _trace:
        import ntff_shim  # noqa: F401
    res = run_bass_kernel_spmd(nc, in_maps, core_ids=list(range(CORES)),
                               trace=_trace)
    LAST_RESULTS = res
    return np.concatenate([res.results[c]["out"] for c in range(CORES)],
                          axis=0)
